# revision 6
# baseline (speedup 1.0000x reference)
"""MaxSim (ColBERT late-interaction) retrieval scoring on Trainium2.

scores[q, d] = sum_m max_n <Q[q,m,:], D[d,n,:]>
Q [32, 32, 128], D [256, 180, 128] -> scores [32, 256] fp32.

Sharding: doc axis split across 8 NeuronCores (32 docs each), full Q
replicated; per-core partial scores [32q, 32d] concatenated on host.

Per-core layout:
  - 128 SBUF partitions = 4 queries x 32 query-tokens (8 query "groups").
  - PE: per group, Q-group [128h, 128qm] is the stationary operand; doc
    tokens stream through as the moving operand -> PSUM sim tiles.
  - DVE: reduce_max over each doc's 180 token columns (per 8-doc block).
  - PE: tiny selector matmul sums each query's 32 token-maxes:
    sel[128,4].T @ maxvals[128, g*d] -> [4, g*d] -> DMA to [32q, 32d].
"""

import numpy as np
import ml_dtypes
from contextlib import ExitStack

import concourse.bass as bass
import concourse.mybir as mybir
import concourse.tile as tile
from concourse import bacc
from concourse.bass_utils import run_bass_kernel_spmd

BF16 = ml_dtypes.bfloat16

H = 128            # head dim (contraction)
NQ, M = 32, 32     # queries, query tokens
NDOC, NTOK = 256, 180
NCORES = 8
DSHARD = NDOC // NCORES      # 32 docs per core
GROUPS, QPG = 8, 4           # query groups of 4; 4*32 = 128 partitions
BLK_DOCS = 8                 # docs per PSUM block
NBLK = DSHARD // BLK_DOCS    # 4 blocks
BLK = BLK_DOCS * NTOK        # 1440 sim columns per block
GCOLS = DSHARD * NTOK        # 5760 doc-token columns per core

_CACHE = {}


def _build():
    nc = bacc.Bacc(None, target_bir_lowering=False)
    qt = nc.dram_tensor("qt", [H, GROUPS * 128], mybir.dt.bfloat16,
                        kind="ExternalInput")
    dt = nc.dram_tensor("dt", [H, GCOLS], mybir.dt.bfloat16,
                        kind="ExternalInput")
    sel = nc.dram_tensor("sel", [H, QPG], mybir.dt.bfloat16,
                         kind="ExternalInput")
    scores = nc.dram_tensor("scores", [NQ, DSHARD], mybir.dt.float32,
                            kind="ExternalOutput")

    with ExitStack() as ctx:
        tc = ctx.enter_context(tile.TileContext(nc))
        singles = ctx.enter_context(tc.tile_pool(name="singles", bufs=1))
        psums = ctx.enter_context(tc.tile_pool(name="psums", bufs=2,
                                               space="PSUM"))
        outp = ctx.enter_context(tc.tile_pool(name="outp", bufs=1,
                                              space="PSUM"))

        q_sb = singles.tile([H, GROUPS * 128], mybir.dt.bfloat16)
        d_sb = singles.tile([H, GCOLS], mybir.dt.bfloat16)
        sel_sb = singles.tile([H, QPG], mybir.dt.bfloat16)
        maxv = singles.tile([128, GROUPS, DSHARD], mybir.dt.bfloat16)

        nc.sync.dma_start(out=q_sb, in_=qt[:, :])
        nc.sync.dma_start(out=sel_sb, in_=sel[:, :])
        for b in range(NBLK):
            nc.sync.dma_start(out=d_sb[:, b * BLK:(b + 1) * BLK],
                              in_=dt[:, b * BLK:(b + 1) * BLK])

        mm_splits = [(0, 512), (512, 512), (1024, BLK - 1024)]
        for g in range(GROUPS):
            qg = q_sb[:, g * 128:(g + 1) * 128]
            for b in range(NBLK):
                ps = psums.tile([128, BLK], mybir.dt.float32, tag="ps")
                for (o, w) in mm_splits:
                    nc.tensor.matmul(ps[:, o:o + w], qg,
                                     d_sb[:, b * BLK + o: b * BLK + o + w],
                                     start=True, stop=True)
                nc.vector.reduce_max(
                    maxv[:, g, b * BLK_DOCS:(b + 1) * BLK_DOCS],
                    ps.rearrange("p (d n) -> p d n", n=NTOK),
                    axis=mybir.AxisListType.X)

        outps = outp.tile([QPG, GROUPS * DSHARD], mybir.dt.float32)
        nc.tensor.matmul(outps, sel_sb,
                         maxv.rearrange("p g d -> p (g d)"),
                         start=True, stop=True)
        scores_sb = singles.tile([QPG, GROUPS * DSHARD], mybir.dt.float32)
        nc.scalar.copy(out=scores_sb, in_=outps)
        nc.sync.dma_start(
            out=scores[:, :].rearrange("(g j) d -> j g d", j=QPG),
            in_=scores_sb.rearrange("j (g d) -> j g d", g=GROUPS))
    nc.finalize()
    return nc


def _get_program():
    if "nc" not in _CACHE:
        _CACHE["nc"] = _build()
    return _CACHE["nc"]


def _prep_inputs(Q, D, q_mask, d_mask):
    Qm = np.asarray(Q, np.float32) * np.asarray(q_mask, np.float32)[..., None]
    Dm = np.asarray(D, np.float32) * np.asarray(d_mask, np.float32)[..., None]

    qt = np.ascontiguousarray(
        Qm.reshape(GROUPS, QPG, M, H).transpose(3, 0, 1, 2).reshape(H, GROUPS * 128)
    ).astype(BF16)
    sel = np.ascontiguousarray(
        np.repeat(np.eye(QPG, dtype=np.float32), M, axis=0)).astype(BF16)

    in_maps = []
    for c in range(NCORES):
        Dc = Dm[c * DSHARD:(c + 1) * DSHARD]          # [32, 180, 128]
        dtc = np.ascontiguousarray(
            Dc.transpose(2, 0, 1).reshape(H, GCOLS)).astype(BF16)
        in_maps.append({"qt": qt, "dt": dtc, "sel": sel})
    return in_maps


def run(Q, D, q_mask, d_mask, trace=False, **spmd_kwargs):
    """Run the sharded kernel; returns (scores [32,256] fp32, BassKernelResults)."""
    nc = _get_program()
    in_maps = _prep_inputs(Q, D, q_mask, d_mask)
    res = run_bass_kernel_spmd(nc, in_maps, core_ids=list(range(NCORES)),
                               trace=trace, **spmd_kwargs)
    full = np.empty((NQ, NDOC), dtype=np.float32)
    for c in range(NCORES):
        full[:, c * DSHARD:(c + 1) * DSHARD] = res.results[c]["scores"]
    return full, res


def kernel(Q, D, q_mask, d_mask):
    out, _ = run(Q, D, q_mask, d_mask, trace=False)
    return out
